# revision 10
# baseline (speedup 1.0000x reference)
"""Causal single-head attention (B=4, S=4096, D=1024) on 8 trn2 NeuronCores.

Sharding: 2 cores per batch element. Each core owns 16 interleaved 128-row
query blocks (core parity k takes global blocks g = 2t + k, t = 0..15), which
balances the causal triangle exactly: local block t attends to key columns
[0, (2t+2)*128), identical extent on every core, so one SPMD program serves
all 8 cores.

Compute strategy (variant "g1", fp8 DoubleRow):
  All heavy matmuls run in fp8e4 with MatmulPerfMode.DoubleRow (pairs of
  128-row contraction chunks per instruction, ~3.7x bf16 throughput on HW).
  Scores are computed TRANSPOSED (keys on the partition axis) so the exp'd
  attention tile is directly consumable as the stationary operand of the PV
  matmul - no PE transposes at all. Softmax row-sums are per-query-column
  sums = partition-axis reductions, done with tiny ones-vector matmuls that
  accumulate over key-block pairs in PSUM; normalization therefore uses the
  QUANTIZED attention weights, cancelling common-mode fp8 error.

  Query blocks are processed in quads (4 query blocks share one 512-wide
  moving operand) to keep matmul instructions >= 128 cycles.

  fp8 noise protection: rows attending to few keys (global rows 0..255,
  i.e. local block t=0) cannot average away fp8 noise, so block t=0 runs a
  bf16 path end-to-end using HOST-precomputed q/k/v stashes (q0/k0/v0).
  Wq/Wk/Wv are pre-scaled by 32 before fp8 quantization to avoid the fp8
  subnormal range (std 0.02 -> 0.64); the 32^2 score scale folds into the
  exp scale and the 32 in v folds into the ones-vector (=32) used for sums.
"""

import math

import numpy as np
import ml_dtypes

from concourse import bacc, mybir, tile
from concourse.bass_utils import run_bass_kernel_spmd

B, S, D = 4, 4096, 1024
NCORES = 8
P = 128
DK = D // P          # 8 contraction chunks of 128
DP = DK // 2         # 4 DoubleRow pair-chunks
NQB = (S // 2) // P  # 16 local query blocks per core
NEG = -1.0e30

_CACHE = {}

F8 = mybir.dt.float8e4
DR = mybir.MatmulPerfMode.DoubleRow


def _build_program_g(reps=1, phase="full", variant="g1"):
    bf16 = mybir.dt.bfloat16
    f32 = mybir.dt.float32
    nc = bacc.Bacc(
        "TRN2",
        target_bir_lowering=False,
        debug=False,
        num_devices=NCORES,
    )

    xq8_d = nc.dram_tensor("xq8", [DP, P, 2, S // 2], F8, kind="ExternalInput")
    xt8_d = nc.dram_tensor("xt8", [DP, P, 2, S], F8, kind="ExternalInput")
    wq8_d = nc.dram_tensor("wq8", [DP, P, 2, D], F8, kind="ExternalInput")
    wk8_d = nc.dram_tensor("wk8", [DP, P, 2, D], F8, kind="ExternalInput")
    wv8_d = nc.dram_tensor("wv8", [DP, P, 2, D], F8, kind="ExternalInput")
    q0_d = nc.dram_tensor("q0", [P, DK, P], bf16, kind="ExternalInput")
    k0_d = nc.dram_tensor("k0", [P, DK, 2 * P], bf16, kind="ExternalInput")
    v0_d = nc.dram_tensor("v0", [2, P, D], bf16, kind="ExternalInput")
    mask_d = nc.dram_tensor("mask", [P, 2, P], bf16, kind="ExternalInput")
    ones8_d = nc.dram_tensor("ones8", [P, 2, 1], F8, kind="ExternalInput")
    onesb_d = nc.dram_tensor("onesb", [P, 1], bf16, kind="ExternalInput")
    g2 = variant >= "g2"
    odt = bf16 if g2 else f32
    out_d = nc.dram_tensor("out", [NQB, P, D], odt, kind="ExternalOutput")

    ESC = 1.0 / (32.0 * 32.0 * 32.0)  # exp scale for 32x-scaled W on both sides

    g5 = variant >= "g5"
    with tile.TileContext(nc) as tc:
        with (
            tc.tile_pool(name="const", bufs=1) as constp,
            tc.tile_pool(name="w", bufs=1) as wp,
            tc.tile_pool(name="slab", bufs=(16 if g2 else 12)) as slabp,
            tc.tile_pool(name="qT", bufs=1) as qTp,
            tc.tile_pool(name="kT", bufs=1) as kTp,
            tc.tile_pool(name="v", bufs=1) as vp,
            tc.tile_pool(name="attn", bufs=(3 if g5 else 2)) as attnp,
            tc.tile_pool(name="a0", bufs=2) as a0p,
            tc.tile_pool(name="stat", bufs=2) as statp,
            tc.tile_pool(name="outst", bufs=(4 if g2 else 2)) as outp,
            tc.tile_pool(name="pss", bufs=(5 if variant == "g6" else (4 if g2 else 3)),
                         space="PSUM") as pssp,
            tc.tile_pool(name="psv", bufs=(2 if variant == "g6" else (3 if g2 else 2)),
                         space="PSUM") as psvp,
            tc.tile_pool(name="pssum", bufs=(1 if g2 else 2), space="PSUM") as pssump,
        ):
            def load_consts():
                mask_t = constp.tile([P, 2, P], bf16, tag="mask", name="mask_t")
                nc.sync.dma_start(mask_t[:], mask_d[:])
                ones8_t = constp.tile([P, 2, 1], F8, tag="ones8", name="ones8_t")
                nc.sync.dma_start(ones8_t[:], ones8_d[:])
                onesb_t = constp.tile([P, 1], bf16, tag="onesb", name="onesb_t")
                nc.sync.dma_start(onesb_t[:], onesb_d[:])
                q0_t = constp.tile([P, DK, P], bf16, tag="q0", name="q0_t")
                nc.sync.dma_start(q0_t[:], q0_d[:])
                k0_t = constp.tile([P, DK, 2 * P], bf16, tag="k0", name="k0_t")
                nc.sync.dma_start(k0_t[:], k0_d[:])
                v0_t = [constp.tile([P, D], bf16, tag=f"v0_{j}", name=f"v0_{j}")
                        for j in range(2)]
                for j in range(2):
                    nc.sync.dma_start(v0_t[j][:], v0_d[j])
                return mask_t, ones8_t, onesb_t, q0_t, k0_t, v0_t

            if not g5:
                # consts loaded before first weights: delays the first matmul
                mask_t, ones8_t, onesb_t, q0_t, k0_t, v0_t = load_consts()

            def load_w(wdram):
                ws = []
                for p in range(DP):
                    w = wp.tile([P, 2, D], F8, tag=f"w{wdram.name}{p}",
                                name=f"w{wdram.name}{p}")
                    nc.sync.dma_start(w[:], wdram[p])
                    ws.append(w)
                return ws

            def load_slab8(src, c0, cw):
                slab = []
                for p in range(DP):
                    t = slabp.tile([P, 2, 512], F8, tag="slab", name=f"slab{p}")
                    nc.sync.dma_start(t[:, :, :cw], src[p][:, :, c0:c0 + cw])
                    slab.append(t)
                return slab

            def evict(dst_ap, ps_ap, idx):
                # PSUM->SBUF eviction engine split, per-variant policy
                if variant == "g3":
                    dve = idx % 2 == 0        # 50/50 DVE:ACT
                elif variant == "g4":
                    dve = idx % 4 != 3        # 3:1 DVE:ACT
                else:
                    dve = g2 or idx % 2 == 0  # g2: all DVE; g1: 50/50
                if dve:
                    nc.vector.tensor_copy(dst_ap, ps_ap)
                else:
                    nc.scalar.copy(dst_ap, ps_ap)

            for rep in range(reps):
                # ---- Q projection (fp8 DoubleRow): qT8[o-pair][:, h, m]
                wq = load_w(wq8_d)
                qT8 = [qTp.tile([P, 2, S // 2], F8, tag=f"qT{i}", name=f"qT{i}")
                       for i in range(DP)]
                for mg in range(4):
                    slab = load_slab8(xq8_d, mg * 512, 512)
                    for o in range(DK):
                        ps = pssp.tile([P, 512], f32, tag="pss", name="ps")
                        for p in range(DP):
                            nc.tensor.matmul(
                                ps[:],
                                wq[p][:, :, o * P:(o + 1) * P],
                                slab[p][:],
                                start=(p == 0), stop=(p == DP - 1),
                                perf_mode=DR,
                            )
                        evict(qT8[o // 2][:, o % 2, mg * 512:(mg + 1) * 512],
                              ps[:], o)

                # ---- K + V projections share xT slabs
                wk = load_w(wk8_d)
                wv = load_w(wv8_d)
                kT8 = [kTp.tile([P, 2, S], F8, tag=f"kT{i}", name=f"kT{i}")
                       for i in range(DP)]
                v8 = [vp.tile([P, 2, D], F8, tag=f"v{i}", name=f"v{i}")
                      for i in range(S // 256)]
                for sg in range(8):
                    slab = load_slab8(xt8_d, sg * 512, 512)
                    for o in range(DK):
                        ps = pssp.tile([P, 512], f32, tag="pss", name="ps")
                        for p in range(DP):
                            nc.tensor.matmul(
                                ps[:],
                                wk[p][:, :, o * P:(o + 1) * P],
                                slab[p][:],
                                start=(p == 0), stop=(p == DP - 1),
                                perf_mode=DR,
                            )
                        evict(kT8[o // 2][:, o % 2, sg * 512:(sg + 1) * 512],
                              ps[:], o)
                    for ss in range(4):
                        j = sg * 4 + ss
                        for h in range(2):
                            ps = pssp.tile([P, 512], f32, tag="pss", name="ps")
                            for p in range(DP):
                                nc.tensor.matmul(
                                    ps[:],
                                    slab[p][:, :, ss * P:(ss + 1) * P],
                                    wv[p][:, :, h * 512:(h + 1) * 512],
                                    start=(p == 0), stop=(p == DP - 1),
                                    perf_mode=DR,
                                )
                            evict(v8[j // 2][:, j % 2, h * 512:(h + 1) * 512],
                                  ps[:], ss + h)

                if phase == "proj":
                    outst = outp.tile([P, D], odt, tag="outst", name="outst")
                    nc.vector.tensor_copy(outst[:, 0:512], qT8[0][:, 0, 0:512])
                    nc.vector.tensor_copy(outst[:, 512:768], kT8[0][:, 0, 0:256])
                    nc.vector.tensor_copy(outst[:, 768:1024], v8[0][:, 0, 0:256])
                    nc.sync.dma_start(out_d[0], outst[:])
                    continue

                # ---- block t=0: bf16 stash path (host q0/k0/v0)
                attnT0 = a0p.tile([P, 2 * P], bf16, tag="attnT0", name="attnT0")
                for j in range(2):
                    ps = pssp.tile([P, 512], f32, tag="pss", name="ps")
                    for o in range(DK):
                        nc.tensor.matmul(
                            ps[:, 0:P],
                            k0_t[:, o, j * P:(j + 1) * P],
                            q0_t[:, o, :],
                            start=(o == 0), stop=(o == DK - 1),
                        )
                    nc.vector.tensor_add(ps[:, 0:P], ps[:, 0:P], mask_t[:, j, :])
                    nc.scalar.activation(
                        attnT0[:, j * P:(j + 1) * P], ps[:, 0:P],
                        mybir.ActivationFunctionType.Exp, scale=1.0 / 32.0,
                    )
                s0 = pssump.tile([P, 1], f32, tag="ssum", name="s0")
                for j in range(2):
                    nc.tensor.matmul(
                        s0[:], attnT0[:, j * P:(j + 1) * P], onesb_t[:],
                        start=(j == 0), stop=(j == 1),
                    )
                rec0 = statp.tile([P, 1], f32, tag="rec0", name="rec0")
                nc.vector.reciprocal(rec0[:], s0[:])

                # ---- attention in quads of 4 query blocks
                def emit_pv(entries):
                    for entry in entries:
                        outst = outp.tile([P, D], odt, tag="outst", name="outst")
                        if entry[0] == "stash":
                            for h in range(2):
                                ps = psvp.tile([P, 512], f32, tag="psv", name="psv")
                                for j in range(2):
                                    nc.tensor.matmul(
                                        ps[:],
                                        attnT0[:, j * P:(j + 1) * P],
                                        v0_t[j][:, h * 512:(h + 1) * 512],
                                        start=(j == 0), stop=(j == 1),
                                    )
                                nc.vector.tensor_scalar_mul(
                                    outst[:, h * 512:(h + 1) * 512], ps[:], rec0[:]
                                )
                            nc.sync.dma_start(out_d[0], outst[:])
                            continue
                        _, t, Tq0, aT0, rec = entry
                        roff = (t - Tq0) * P
                        for h in range(2):
                            ps = psvp.tile([P, 512], f32, tag="psv", name="psv")
                            for m in range(t + 1):
                                nc.tensor.matmul(
                                    ps[:],
                                    aT0[m][:, :, roff:roff + P],
                                    v8[m][:, :, h * 512:(h + 1) * 512],
                                    start=(m == 0), stop=(m == t),
                                    perf_mode=DR,
                                )
                            nc.vector.tensor_scalar_mul(
                                outst[:, h * 512:(h + 1) * 512], ps[:], rec[:]
                            )
                        nc.sync.dma_start(out_d[t], outst[:])

                prev = None
                for Tq in (0, 4, 8, 12):
                    aT = [attnp.tile([P, 2, 512], F8, tag=f"aT{m}", name=f"aT{m}")
                          for m in range(Tq + 4)]
                    # scores for query cols [Tq*128, (Tq+4)*128)
                    for m in range(Tq + 4):
                        start_t = max(m, Tq + 1) if Tq == 0 else max(m, Tq)
                        q_off = (start_t - Tq) * P
                        w = 512 - q_off
                        for jj in range(2):
                            jcol = (2 * m + jj) * P
                            ps = pssp.tile([P, 512], f32, tag="pss", name="ps")
                            for p in range(DP):
                                nc.tensor.matmul(
                                    ps[:, 0:w],
                                    kT8[p][:, :, jcol:jcol + P],
                                    qT8[p][:, :, Tq * P + q_off:(Tq + 4) * P],
                                    start=(p == 0), stop=(p == DP - 1),
                                    perf_mode=DR,
                                )
                            if start_t == m:
                                nc.vector.tensor_add(
                                    ps[:, 0:P], ps[:, 0:P], mask_t[:, jj, :]
                                )
                            nc.scalar.activation(
                                aT[m][:, jj, q_off:512], ps[:, 0:w],
                                mybir.ActivationFunctionType.Exp, scale=ESC,
                            )
                    if prev is not None:
                        emit_pv(prev)
                    # sums + recips for this quad
                    cur = []
                    for t in range(Tq, Tq + 4):
                        if t == 0:
                            cur.append(("stash",))
                            continue
                        s = pssump.tile([P, 1], f32, tag="ssum", name="s")
                        roff = (t - Tq) * P
                        for m in range(t + 1):
                            nc.tensor.matmul(
                                s[:],
                                aT[m][:, :, roff:roff + P],
                                ones8_t[:],
                                start=(m == 0), stop=(m == t),
                                perf_mode=DR,
                            )
                        rec = statp.tile([P, 1], f32, tag=f"rec{t - Tq}",
                                         name=f"rec{t - Tq}")
                        nc.vector.reciprocal(rec[:], s[:])
                        cur.append(("pv", t, Tq, aT, rec))
                    if phase == "scores":
                        # dump a sliver instead of PV
                        for entry in cur:
                            if entry[0] != "pv":
                                continue
                            _, t, Tq0, aT0, rec = entry
                            outst = outp.tile([P, D], odt, tag="outst",
                                              name="outst")
                            roff = (t - Tq0) * P
                            nc.vector.tensor_copy(
                                outst[:, 0:P], aT0[0][:, 0, roff:roff + P])
                            nc.vector.tensor_copy(outst[:, P:P + 1], rec[:])
                            nc.sync.dma_start(out_d[t], outst[:])
                        prev = None
                        continue
                    prev = cur
                if phase == "full":
                    emit_pv(prev)

    nc.compile()
    return nc


def _get_program(reps=1, phase="full", variant="g4"):
    key = ("nc", reps, phase, variant)
    if key not in _CACHE:
        _CACHE[key] = _build_program_g(reps, phase, variant)
    return _CACHE[key]


def _pair_layout(a):
    """[R=1024, C] fp32 -> [DP, P, 2, C] fp8 pair-chunk layout."""
    f8 = ml_dtypes.float8_e4m3
    C = a.shape[1]
    return np.ascontiguousarray(
        np.clip(a, -240.0, 240.0).reshape(DP, 2, P, C).transpose(0, 2, 1, 3)
    ).astype(f8)


def _make_in_maps(x, Wq, Wk, Wv):
    bf16 = ml_dtypes.bfloat16
    f8 = ml_dtypes.float8_e4m3

    wq8 = _pair_layout(32.0 * Wq.T)
    wk8 = _pair_layout(32.0 * Wk.T)
    wv8 = _pair_layout(32.0 * Wv.T)
    ones8 = np.full((P, 2, 1), 32.0, np.float32).astype(f8)
    onesb = np.ones((P, 1), np.float32).astype(bf16)

    masks = []
    tri = np.triu(np.full((P, P), NEG, np.float32), k=1).T  # [r,c]=NEG where r>c
    for k in range(2):
        m = np.zeros((P, 2, P), np.float32)
        if k == 0:
            m[:, 0, :] = tri
            m[:, 1, :] = NEG
        else:
            m[:, 1, :] = tri
        masks.append(m.astype(bf16))

    in_maps = []
    for c in range(NCORES):
        b, k = c // 2, c % 2
        xb = x[b]                      # [S, D]
        xb_T = np.ascontiguousarray(xb.T)  # [D, S]
        q_cols = np.concatenate(
            [np.arange((2 * t + k) * P, (2 * t + k + 1) * P) for t in range(NQB)]
        )
        xq8 = _pair_layout(np.ascontiguousarray(xb_T[:, q_cols]))
        xt8 = _pair_layout(xb_T)

        # host bf16 stash for block t=0 (query rows k*128..k*128+127,
        # key/value rows 0..255)
        qrows = xb[k * P:(k + 1) * P]            # [P, D]
        q_loc = qrows @ Wq.T                     # [P, D]
        q0 = np.ascontiguousarray(
            q_loc.T.reshape(DK, P, P).transpose(1, 0, 2)).astype(bf16)
        krows = xb[0:2 * P]
        k_loc = krows @ Wk.T                     # [2P, D]
        k0 = np.ascontiguousarray(
            k_loc.T.reshape(DK, P, 2 * P).transpose(1, 0, 2)).astype(bf16)
        v_loc = krows @ Wv.T                     # [2P, D]
        v0 = np.ascontiguousarray(v_loc.reshape(2, P, D)).astype(bf16)

        in_maps.append(
            {
                "xq8": xq8,
                "xt8": xt8,
                "wq8": wq8,
                "wk8": wk8,
                "wv8": wv8,
                "q0": q0,
                "k0": k0,
                "v0": v0,
                "mask": masks[k],
                "ones8": ones8,
                "onesb": onesb,
            }
        )
    return in_maps


def kernel(x, Wq, Wk, Wv):
    x = np.asarray(x, dtype=np.float32)
    Wq = np.asarray(Wq, dtype=np.float32)
    Wk = np.asarray(Wk, dtype=np.float32)
    Wv = np.asarray(Wv, dtype=np.float32)

    nc = _get_program()
    in_maps = _make_in_maps(x, Wq, Wk, Wv)
    try:
        res = run_bass_kernel_spmd(nc, in_maps, list(range(NCORES)))
    except ModuleNotFoundError:
        # profiling hook unavailable in this environment; run untraced
        import os as _os
        _os.environ["BASS_NEVER_TRACE"] = "1"
        res = run_bass_kernel_spmd(nc, in_maps, list(range(NCORES)))

    out = np.empty((B, S, D), np.float32)
    for c in range(NCORES):
        b, k = c // 2, c % 2
        oc = res.results[c]["out"]  # [NQB, P, D]
        for t in range(NQB):
            g = 2 * t + k
            out[b, g * P:(g + 1) * P, :] = oc[t]
    return out


# revision 12
# speedup vs baseline: 1.9093x; 1.9093x over previous
"""Causal single-head attention (B=4, S=4096, D=1024) on 8 trn2 NeuronCores.

Sharding: 2 cores per batch element. Each core owns 16 interleaved 128-row
query blocks (core parity k takes global blocks g = 2t + k, t = 0..15), which
balances the causal triangle exactly: local block t attends to key columns
[0, (2t+2)*128), identical extent on every core, so one SPMD program serves
all 8 cores.

Compute strategy (variant "g1", fp8 DoubleRow):
  All heavy matmuls run in fp8e4 with MatmulPerfMode.DoubleRow (pairs of
  128-row contraction chunks per instruction, ~3.7x bf16 throughput on HW).
  Scores are computed TRANSPOSED (keys on the partition axis) so the exp'd
  attention tile is directly consumable as the stationary operand of the PV
  matmul - no PE transposes at all. Softmax row-sums are per-query-column
  sums = partition-axis reductions, done with tiny ones-vector matmuls that
  accumulate over key-block pairs in PSUM; normalization therefore uses the
  QUANTIZED attention weights, cancelling common-mode fp8 error.

  Query blocks are processed in quads (4 query blocks share one 512-wide
  moving operand) to keep matmul instructions >= 128 cycles.

  fp8 noise protection: rows attending to few keys (global rows 0..255,
  i.e. local block t=0) cannot average away fp8 noise, so block t=0 runs a
  bf16 path end-to-end using HOST-precomputed q/k/v stashes (q0/k0/v0).
  Wq/Wk/Wv are pre-scaled by 32 before fp8 quantization to avoid the fp8
  subnormal range (std 0.02 -> 0.64); the 32^2 score scale folds into the
  exp scale and the 32 in v folds into the ones-vector (=32) used for sums.
"""

import math

import numpy as np
import ml_dtypes

from concourse import bacc, mybir, tile
from concourse.bass_utils import run_bass_kernel_spmd

B, S, D = 4, 4096, 1024
NCORES = 8
P = 128
DK = D // P          # 8 contraction chunks of 128
DP = DK // 2         # 4 DoubleRow pair-chunks
NQB = (S // 2) // P  # 16 local query blocks per core
NEG = -1.0e30

_CACHE = {}

F8 = mybir.dt.float8e4
DR = mybir.MatmulPerfMode.DoubleRow


def _build_program_g(reps=1, phase="full", variant="g1"):
    bf16 = mybir.dt.bfloat16
    f32 = mybir.dt.float32
    nc = bacc.Bacc(
        "TRN2",
        target_bir_lowering=False,
        debug=False,
        num_devices=NCORES,
    )

    xq8_d = nc.dram_tensor("xq8", [DP, P, 2, S // 2], F8, kind="ExternalInput")
    xt8_d = nc.dram_tensor("xt8", [DP, P, 2, S], F8, kind="ExternalInput")
    wq8_d = nc.dram_tensor("wq8", [DP, P, 2, D], F8, kind="ExternalInput")
    wk8_d = nc.dram_tensor("wk8", [DP, P, 2, D], F8, kind="ExternalInput")
    wv8_d = nc.dram_tensor("wv8", [DP, P, 2, D], F8, kind="ExternalInput")
    q0_d = nc.dram_tensor("q0", [P, DK, P], bf16, kind="ExternalInput")
    k0_d = nc.dram_tensor("k0", [P, DK, 2 * P], bf16, kind="ExternalInput")
    v0_d = nc.dram_tensor("v0", [2, P, D], bf16, kind="ExternalInput")
    mask_d = nc.dram_tensor("mask", [P, 2, P], bf16, kind="ExternalInput")
    ones8_d = nc.dram_tensor("ones8", [P, 2, 1], F8, kind="ExternalInput")
    onesb_d = nc.dram_tensor("onesb", [P, 1], bf16, kind="ExternalInput")
    g2 = variant >= "g2"
    odt = bf16 if g2 else f32
    out_d = nc.dram_tensor("out", [NQB, P, D], odt, kind="ExternalOutput")

    ESC = 1.0 / (32.0 * 32.0 * 32.0)  # exp scale for 32x-scaled W on both sides

    g5 = variant >= "g5"
    with tile.TileContext(nc) as tc:
        with (
            tc.tile_pool(name="const", bufs=1) as constp,
            tc.tile_pool(name="w", bufs=1) as wp,
            tc.tile_pool(name="slab", bufs=(16 if g2 else 12)) as slabp,
            tc.tile_pool(name="qT", bufs=1) as qTp,
            tc.tile_pool(name="kT", bufs=1) as kTp,
            tc.tile_pool(name="v", bufs=1) as vp,
            tc.tile_pool(name="attn", bufs=(3 if g5 else 2)) as attnp,
            tc.tile_pool(name="a0", bufs=2) as a0p,
            tc.tile_pool(name="stat", bufs=2) as statp,
            tc.tile_pool(name="outst", bufs=(4 if g2 else 2)) as outp,
            tc.tile_pool(name="pss", bufs=(5 if variant == "g6" else (4 if g2 else 3)),
                         space="PSUM") as pssp,
            tc.tile_pool(name="psv", bufs=(2 if variant == "g6" else (3 if g2 else 2)),
                         space="PSUM") as psvp,
            tc.tile_pool(name="pssum", bufs=(1 if g2 else 2), space="PSUM") as pssump,
        ):
            def load_consts():
                mask_t = constp.tile([P, 2, P], bf16, tag="mask", name="mask_t")
                nc.sync.dma_start(mask_t[:], mask_d[:])
                ones8_t = constp.tile([P, 2, 1], F8, tag="ones8", name="ones8_t")
                nc.sync.dma_start(ones8_t[:], ones8_d[:])
                onesb_t = constp.tile([P, 1], bf16, tag="onesb", name="onesb_t")
                nc.sync.dma_start(onesb_t[:], onesb_d[:])
                q0_t = constp.tile([P, DK, P], bf16, tag="q0", name="q0_t")
                nc.sync.dma_start(q0_t[:], q0_d[:])
                k0_t = constp.tile([P, DK, 2 * P], bf16, tag="k0", name="k0_t")
                nc.sync.dma_start(k0_t[:], k0_d[:])
                v0_t = [constp.tile([P, D], bf16, tag=f"v0_{j}", name=f"v0_{j}")
                        for j in range(2)]
                for j in range(2):
                    nc.sync.dma_start(v0_t[j][:], v0_d[j])
                return mask_t, ones8_t, onesb_t, q0_t, k0_t, v0_t

            if not g5:
                # consts loaded before first weights: delays the first matmul
                mask_t, ones8_t, onesb_t, q0_t, k0_t, v0_t = load_consts()

            def load_w(wdram):
                ws = []
                for p in range(DP):
                    w = wp.tile([P, 2, D], F8, tag=f"w{wdram.name}{p}",
                                name=f"w{wdram.name}{p}")
                    nc.sync.dma_start(w[:], wdram[p])
                    ws.append(w)
                return ws

            def load_slab8(src, c0, cw):
                slab = []
                for p in range(DP):
                    t = slabp.tile([P, 2, 512], F8, tag="slab", name=f"slab{p}")
                    nc.sync.dma_start(t[:, :, :cw], src[p][:, :, c0:c0 + cw])
                    slab.append(t)
                return slab

            def evict(dst_ap, ps_ap, idx):
                # PSUM->SBUF eviction engine split, per-variant policy
                if variant == "g3":
                    dve = idx % 2 == 0        # 50/50 DVE:ACT
                elif variant == "g4":
                    dve = idx % 4 != 3        # 3:1 DVE:ACT
                else:
                    dve = g2 or idx % 2 == 0  # g2: all DVE; g1: 50/50
                if dve:
                    nc.vector.tensor_copy(dst_ap, ps_ap)
                else:
                    nc.scalar.copy(dst_ap, ps_ap)

            for rep in range(reps):
                # ---- Q projection (fp8 DoubleRow): qT8[o-pair][:, h, m]
                wq = load_w(wq8_d)
                if g5 and rep == 0:
                    # consts after the first weight DMAs: the first matmul
                    # only waits for wq + slab 0
                    mask_t, ones8_t, onesb_t, q0_t, k0_t, v0_t = load_consts()
                qT8 = [qTp.tile([P, 2, S // 2], F8, tag=f"qT{i}", name=f"qT{i}")
                       for i in range(DP)]
                for mg in range(4):
                    slab = load_slab8(xq8_d, mg * 512, 512)
                    for o in range(DK):
                        ps = pssp.tile([P, 512], f32, tag="pss", name="ps")
                        for p in range(DP):
                            nc.tensor.matmul(
                                ps[:],
                                wq[p][:, :, o * P:(o + 1) * P],
                                slab[p][:],
                                start=(p == 0), stop=(p == DP - 1),
                                perf_mode=DR,
                            )
                        evict(qT8[o // 2][:, o % 2, mg * 512:(mg + 1) * 512],
                              ps[:], o)

                # ---- K + V projections share xT slabs
                wk = load_w(wk8_d)
                wv = load_w(wv8_d)
                kT8 = [kTp.tile([P, 2, S], F8, tag=f"kT{i}", name=f"kT{i}")
                       for i in range(DP)]
                v8 = [vp.tile([P, 2, D], F8, tag=f"v{i}", name=f"v{i}")
                      for i in range(S // 256)]
                for sg in range(8):
                    slab = load_slab8(xt8_d, sg * 512, 512)
                    for o in range(DK):
                        ps = pssp.tile([P, 512], f32, tag="pss", name="ps")
                        for p in range(DP):
                            nc.tensor.matmul(
                                ps[:],
                                wk[p][:, :, o * P:(o + 1) * P],
                                slab[p][:],
                                start=(p == 0), stop=(p == DP - 1),
                                perf_mode=DR,
                            )
                        evict(kT8[o // 2][:, o % 2, sg * 512:(sg + 1) * 512],
                              ps[:], o)
                    for ss in range(4):
                        j = sg * 4 + ss
                        for h in range(2):
                            ps = pssp.tile([P, 512], f32, tag="pss", name="ps")
                            for p in range(DP):
                                nc.tensor.matmul(
                                    ps[:],
                                    slab[p][:, :, ss * P:(ss + 1) * P],
                                    wv[p][:, :, h * 512:(h + 1) * 512],
                                    start=(p == 0), stop=(p == DP - 1),
                                    perf_mode=DR,
                                )
                            evict(v8[j // 2][:, j % 2, h * 512:(h + 1) * 512],
                                  ps[:], ss + h)

                if phase == "proj":
                    outst = outp.tile([P, D], odt, tag="outst", name="outst")
                    nc.vector.tensor_copy(outst[:, 0:512], qT8[0][:, 0, 0:512])
                    nc.vector.tensor_copy(outst[:, 512:768], kT8[0][:, 0, 0:256])
                    nc.vector.tensor_copy(outst[:, 768:1024], v8[0][:, 0, 0:256])
                    nc.sync.dma_start(out_d[0], outst[:])
                    continue

                # ---- block t=0: bf16 stash path (host q0/k0/v0)
                attnT0 = a0p.tile([P, 2 * P], bf16, tag="attnT0", name="attnT0")
                for j in range(2):
                    ps = pssp.tile([P, 512], f32, tag="pss", name="ps")
                    for o in range(DK):
                        nc.tensor.matmul(
                            ps[:, 0:P],
                            k0_t[:, o, j * P:(j + 1) * P],
                            q0_t[:, o, :],
                            start=(o == 0), stop=(o == DK - 1),
                        )
                    nc.vector.tensor_add(ps[:, 0:P], ps[:, 0:P], mask_t[:, j, :])
                    nc.scalar.activation(
                        attnT0[:, j * P:(j + 1) * P], ps[:, 0:P],
                        mybir.ActivationFunctionType.Exp, scale=1.0 / 32.0,
                    )
                s0 = pssump.tile([P, 1], f32, tag="ssum", name="s0")
                for j in range(2):
                    nc.tensor.matmul(
                        s0[:], attnT0[:, j * P:(j + 1) * P], onesb_t[:],
                        start=(j == 0), stop=(j == 1),
                    )
                rec0 = statp.tile([P, 1], f32, tag="rec0", name="rec0")
                nc.vector.reciprocal(rec0[:], s0[:])

                # ---- attention in quads of 4 query blocks
                def emit_pv(entries):
                    for entry in entries:
                        outst = outp.tile([P, D], odt, tag="outst", name="outst")
                        if entry[0] == "stash":
                            for h in range(2):
                                ps = psvp.tile([P, 512], f32, tag="psv", name="psv")
                                for j in range(2):
                                    nc.tensor.matmul(
                                        ps[:],
                                        attnT0[:, j * P:(j + 1) * P],
                                        v0_t[j][:, h * 512:(h + 1) * 512],
                                        start=(j == 0), stop=(j == 1),
                                    )
                                nc.vector.tensor_scalar_mul(
                                    outst[:, h * 512:(h + 1) * 512], ps[:], rec0[:]
                                )
                            nc.sync.dma_start(out_d[0], outst[:])
                            continue
                        _, t, Tq0, aT0, rec = entry
                        roff = (t - Tq0) * P
                        for h in range(2):
                            ps = psvp.tile([P, 512], f32, tag="psv", name="psv")
                            for m in range(t + 1):
                                nc.tensor.matmul(
                                    ps[:],
                                    aT0[m][:, :, roff:roff + P],
                                    v8[m][:, :, h * 512:(h + 1) * 512],
                                    start=(m == 0), stop=(m == t),
                                    perf_mode=DR,
                                )
                            if g5 and h == 1:
                                # split rescales DVE/ACT
                                nc.scalar.activation(
                                    outst[:, h * 512:(h + 1) * 512], ps[:],
                                    mybir.ActivationFunctionType.Identity,
                                    scale=rec[:],
                                )
                            else:
                                nc.vector.tensor_scalar_mul(
                                    outst[:, h * 512:(h + 1) * 512], ps[:], rec[:]
                                )
                        nc.sync.dma_start(out_d[t], outst[:])

                prev = None
                for Tq in (0, 4, 8, 12):
                    aT = [attnp.tile([P, 2, 512], F8, tag=f"aT{m}", name=f"aT{m}")
                          for m in range(Tq + 4)]
                    # scores for query cols [Tq*128, (Tq+4)*128)
                    for m in range(Tq + 4):
                        start_t = max(m, Tq + 1) if Tq == 0 else max(m, Tq)
                        q_off = (start_t - Tq) * P
                        w = 512 - q_off
                        for jj in range(2):
                            jcol = (2 * m + jj) * P
                            ps = pssp.tile([P, 512], f32, tag="pss", name="ps")
                            for p in range(DP):
                                nc.tensor.matmul(
                                    ps[:, 0:w],
                                    kT8[p][:, :, jcol:jcol + P],
                                    qT8[p][:, :, Tq * P + q_off:(Tq + 4) * P],
                                    start=(p == 0), stop=(p == DP - 1),
                                    perf_mode=DR,
                                )
                            if start_t == m:
                                nc.vector.tensor_add(
                                    ps[:, 0:P], ps[:, 0:P], mask_t[:, jj, :]
                                )
                            nc.scalar.activation(
                                aT[m][:, jj, q_off:512], ps[:, 0:w],
                                mybir.ActivationFunctionType.Exp, scale=ESC,
                            )
                    if prev is not None:
                        emit_pv(prev)
                    # sums + recips for this quad
                    cur = []
                    for t in range(Tq, Tq + 4):
                        if t == 0:
                            cur.append(("stash",))
                            continue
                        s = pssump.tile([P, 1], f32, tag="ssum", name="s")
                        roff = (t - Tq) * P
                        for m in range(t + 1):
                            nc.tensor.matmul(
                                s[:],
                                aT[m][:, :, roff:roff + P],
                                ones8_t[:],
                                start=(m == 0), stop=(m == t),
                                perf_mode=DR,
                            )
                        rec = statp.tile([P, 1], f32, tag=f"rec{t - Tq}",
                                         name=f"rec{t - Tq}")
                        nc.vector.reciprocal(rec[:], s[:])
                        cur.append(("pv", t, Tq, aT, rec))
                    if phase == "scores":
                        # dump a sliver instead of PV
                        for entry in cur:
                            if entry[0] != "pv":
                                continue
                            _, t, Tq0, aT0, rec = entry
                            outst = outp.tile([P, D], odt, tag="outst",
                                              name="outst")
                            roff = (t - Tq0) * P
                            nc.vector.tensor_copy(
                                outst[:, 0:P], aT0[0][:, 0, roff:roff + P])
                            nc.vector.tensor_copy(outst[:, P:P + 1], rec[:])
                            nc.sync.dma_start(out_d[t], outst[:])
                        prev = None
                        continue
                    prev = cur
                if phase == "full":
                    emit_pv(prev)

    nc.compile()
    return nc


def _get_program(reps=1, phase="full", variant="g4"):
    key = ("nc", reps, phase, variant)
    if key not in _CACHE:
        _CACHE[key] = _build_program_g(reps, phase, variant)
    return _CACHE[key]


def _pair_layout(a):
    """[R=1024, C] fp32 -> [DP, P, 2, C] fp8 pair-chunk layout."""
    f8 = ml_dtypes.float8_e4m3
    C = a.shape[1]
    return np.ascontiguousarray(
        np.clip(a, -240.0, 240.0).reshape(DP, 2, P, C).transpose(0, 2, 1, 3)
    ).astype(f8)


def _make_in_maps(x, Wq, Wk, Wv):
    bf16 = ml_dtypes.bfloat16
    f8 = ml_dtypes.float8_e4m3

    wq8 = _pair_layout(32.0 * Wq.T)
    wk8 = _pair_layout(32.0 * Wk.T)
    wv8 = _pair_layout(32.0 * Wv.T)
    ones8 = np.full((P, 2, 1), 32.0, np.float32).astype(f8)
    onesb = np.ones((P, 1), np.float32).astype(bf16)

    masks = []
    tri = np.triu(np.full((P, P), NEG, np.float32), k=1).T  # [r,c]=NEG where r>c
    for k in range(2):
        m = np.zeros((P, 2, P), np.float32)
        if k == 0:
            m[:, 0, :] = tri
            m[:, 1, :] = NEG
        else:
            m[:, 1, :] = tri
        masks.append(m.astype(bf16))

    in_maps = []
    for c in range(NCORES):
        b, k = c // 2, c % 2
        xb = x[b]                      # [S, D]
        xb_T = np.ascontiguousarray(xb.T)  # [D, S]
        q_cols = np.concatenate(
            [np.arange((2 * t + k) * P, (2 * t + k + 1) * P) for t in range(NQB)]
        )
        xq8 = _pair_layout(np.ascontiguousarray(xb_T[:, q_cols]))
        xt8 = _pair_layout(xb_T)

        # host bf16 stash for block t=0 (query rows k*128..k*128+127,
        # key/value rows 0..255)
        qrows = xb[k * P:(k + 1) * P]            # [P, D]
        q_loc = qrows @ Wq.T                     # [P, D]
        q0 = np.ascontiguousarray(
            q_loc.T.reshape(DK, P, P).transpose(1, 0, 2)).astype(bf16)
        krows = xb[0:2 * P]
        k_loc = krows @ Wk.T                     # [2P, D]
        k0 = np.ascontiguousarray(
            k_loc.T.reshape(DK, P, 2 * P).transpose(1, 0, 2)).astype(bf16)
        v_loc = krows @ Wv.T                     # [2P, D]
        v0 = np.ascontiguousarray(v_loc.reshape(2, P, D)).astype(bf16)

        in_maps.append(
            {
                "xq8": xq8,
                "xt8": xt8,
                "wq8": wq8,
                "wk8": wk8,
                "wv8": wv8,
                "q0": q0,
                "k0": k0,
                "v0": v0,
                "mask": masks[k],
                "ones8": ones8,
                "onesb": onesb,
            }
        )
    return in_maps


def kernel(x, Wq, Wk, Wv):
    x = np.asarray(x, dtype=np.float32)
    Wq = np.asarray(Wq, dtype=np.float32)
    Wk = np.asarray(Wk, dtype=np.float32)
    Wv = np.asarray(Wv, dtype=np.float32)

    nc = _get_program()
    in_maps = _make_in_maps(x, Wq, Wk, Wv)
    try:
        res = run_bass_kernel_spmd(nc, in_maps, list(range(NCORES)))
    except ModuleNotFoundError:
        # profiling hook unavailable in this environment; run untraced
        import os as _os
        _os.environ["BASS_NEVER_TRACE"] = "1"
        res = run_bass_kernel_spmd(nc, in_maps, list(range(NCORES)))

    out = np.empty((B, S, D), np.float32)
    for c in range(NCORES):
        b, k = c // 2, c % 2
        oc = res.results[c]["out"]  # [NQB, P, D]
        for t in range(NQB):
            g = 2 * t + k
            out[b, g * P:(g + 1) * P, :] = oc[t]
    return out
